# revision 29
# baseline (speedup 1.0000x reference)
"""Trainium2 Bass kernel for nn_AutoregressiveRoutingHead (v3).

Model (per batch row b):
    tok_in = [START, tgt[0..6]]                       # teacher forcing, START=5
    x_t    = emb[tok_in[t]]                           # (HID,)
    gi     = x_t @ W_ih.T + b_ih                      # (768,)
    gh     = h @ W_hh.T + b_hh                        # (768,)
    r = sigmoid(gi_r + gh_r); z = sigmoid(gi_z + gh_z)
    n = tanh(gi_n + r * gh_n)
    h' = (1-z)*n + z*h = h + (1-z)*(n - h)
    logits_t = h' @ W_out.T + b_out                   # (5,)

v3 strategy (pure data parallel over batch, 65536 -> 8 x 8192; per core
8192 -> 16 column chunks of 512, processed P=4 at a time as a software
pipeline):

- Host precomputes the token one-hot (incl START at t=0), the transposed f16
  initial hidden state, and gathered n-gate inputs
  in16[b,t] = emb[tok_in[b,t]] @ W_ih_n.T + b_ih_n.
- z-gate weights/tables are NEGATED on the host so sigmoid directly yields
  z' = 1 - z, giving h' = h + z'*(n - h).
- b_hh (r/z part) rides row 6 of the one-hot (always 1.0) through the K=8
  gather matmul; no activation biases needed anywhere.
- Each chunk-step is emitted in 3 phases across pipeline slots so no engine
  queue head-of-line blocks on the serial GRU chain:
    ph1(s):  rz matmuls + 2 sigmoids + hn matmuls + p = r*gh_n + npre = p+i_n
    ph2(s+1): tanh + d = n-h + e = z'*d (GpSimd)
    ph3(s+2): h' = h+e + logits matmul (+ PSUM->SBUF copy / DMA every 4 slots)
  With P=4 interleaved chunks the ~10us chain hides under the ~3.5us/slot
  engine throughput bound.
- Logits: W_out is zero-padded to 32 rows; 4 consecutive slots write one
  PSUM bank at column groups 0/32/64/96, drained by one DVE copy + one DMA
  per 4 slots.
- PSUM: rz pool 2x2 banks + hn 1x2 banks + lg 2x1 bank = 8 banks.
"""

import numpy as np

import concourse.bass as bass
import concourse.mybir as mybir
import concourse.tile as tile
from concourse import bacc, bass_utils

F32 = mybir.dt.float32
F16 = mybir.dt.float16
AF = mybir.ActivationFunctionType
ALU = mybir.AluOpType

N_CORES = 8
B = 65536
L = 8
LATENT = 256
HID = 128
NTOK = 5
V = NTOK + 1  # vocab incl <start>
START = NTOK
G = 3 * LATENT  # 768 gate rows
KC = LATENT // 128  # 2 contraction chunks

B_CORE = B // N_CORES
N_B = 512
P_MAX = 4  # chunks interleaved in the software pipeline


def build_program(b_core=B_CORE, n_b=N_B, use_bhh_n=False, use_bout=False):
    """Build + compile the per-core Bass program (SPMD: same program, 8 cores)."""
    nc = bacc.Bacc("TRN2", target_bir_lowering=False, debug=False)
    n_chunks = b_core // n_b
    P = min(P_MAX, n_chunks)
    assert n_chunks % P == 0
    n_items = n_chunks * L
    assert n_items % 4 == 0
    n_packs = n_items // 4

    # ---- DRAM I/O ----------------------------------------------------------
    latT = nc.dram_tensor("latT", [n_chunks, 128, KC, n_b], F16, kind="ExternalInput").ap()
    oh = nc.dram_tensor("oh", [n_chunks, 8, L, n_b], F16, kind="ExternalInput").ap()
    in16 = nc.dram_tensor("in16", [n_chunks, 128, L, KC, n_b], F16, kind="ExternalInput").ap()
    whhT = nc.dram_tensor("whhT", [128, KC, G], F16, kind="ExternalInput").ap()
    woutT = nc.dram_tensor("woutT", [128, KC, 32], F16, kind="ExternalInput").ap()
    girz = nc.dram_tensor("girz", [128, 2, 128], F16, kind="ExternalInput").ap()
    bhhn = bout = None
    if use_bhh_n:
        bhhn = nc.dram_tensor("bhhn", [1, LATENT], F16, kind="ExternalInput").ap()
    if use_bout:
        bout = nc.dram_tensor("bout", [1, 32], F16, kind="ExternalInput").ap()
    # logits: pipeline slot i -> pack i//4, rows 32*(i%4)+v
    outT = nc.dram_tensor("outT", [n_packs, 128, n_b], F16, kind="ExternalOutput").ap()

    with tile.TileContext(nc) as tc:
        with tc.tile_pool(name="singles", bufs=1) as singles, \
             tc.tile_pool(name="hpool", bufs=4) as h_pool, \
             tc.tile_pool(name="ohpool", bufs=1) as oh_pool, \
             tc.tile_pool(name="inpool", bufs=2) as in_pool, \
             tc.tile_pool(name="gates", bufs=1) as g_pool, \
             tc.tile_pool(name="lgpool", bufs=2) as lg_pool, \
             tc.tile_pool(name="ps_rz", bufs=2, space="PSUM") as ps_rz, \
             tc.tile_pool(name="ps_hn", bufs=1, space="PSUM") as ps_hn, \
             tc.tile_pool(name="ps_lg", bufs=2, space="PSUM") as ps_lg:

            # ---- constants / weights in SBUF -------------------------------
            whh_sb = singles.tile([128, KC, G], F16, tag="whh")
            nc.sync.dma_start(whh_sb, whhT)
            wout_sb = singles.tile([128, KC, 32], F16, tag="wout")
            nc.sync.dma_start(wout_sb, woutT)
            girz_sb = singles.tile([128, 2, 128], F16, tag="girz")
            nc.sync.dma_start(girz_sb, girz)
            bhhn_sb = bout_sb = ones_row = None
            if use_bhh_n or use_bout:
                ones_row = singles.tile([1, n_b], F16, tag="ones_row")
                nc.vector.memset(ones_row, 1.0)
            if use_bhh_n:
                bhhn_sb = singles.tile([1, LATENT], F16, tag="bhhn")
                nc.sync.dma_start(bhhn_sb, bhhn)
            if use_bout:
                bout_sb = singles.tile([1, 32], F16, tag="bout")
                nc.sync.dma_start(bout_sb, bout)

            def chunk_prologue(c, par):
                h = h_pool.tile([128, KC, n_b], F16, tag=f"h{par}", name="h0")
                nc.sync.dma_start(h, latT[c])
                # oh/in16 split into 4 quarter-tiles (2 steps each) for finer
                # SBUF ring recycling (cross-group prefetch) + DMA spreading
                ohq = []
                for q in range(2):
                    oq = oh_pool.tile([128, 4, n_b], F16, tag=f"oh{par}",
                                      name=f"oh_h{q}")
                    for g in range(3):
                        nc.sync.dma_start(oq[32 * g:32 * g + 8],
                                          oh[c, :, 4 * q:4 * q + 4])
                    ohq.append(oq)
                iq = []
                for q in range(4):
                    tq = in_pool.tile([128, 2, KC, n_b], F16, tag=f"in{par}",
                                      name=f"in_q{q}")
                    nc.sync.dma_start(tq, in16[c, :, 2 * q:2 * q + 2])
                    iq.append(tq)
                return {"ohq": ohq, "iq": iq, "h": h}

            # ---- pipeline phases ------------------------------------------
            def ph1(st, t, par, mid_cb=None):
                # All 4 K=8 gather matmuls first (row groups 0/32/64 let the
                # pairs co-stream), then the K=128 recurrence chains (same-
                # group back-to-back matmuls hide their LDWEIGHTS).
                h = st["h"]
                oh_t = st["ohq"][t // 4]
                rz1 = ps_rz.tile([128, 2, n_b], F32, tag="rz", name="rz1")
                rz2 = ps_rz.tile([128, 2, n_b], F32, tag="rz", name="rz2")
                tgts = [rz1[:, 0, :], rz1[:, 1, :], rz2[:, 0, :], rz2[:, 1, :]]
                for m in range(4):
                    if m < 3:
                        gi_lhs = girz_sb[32 * m:32 * m + 8, 0, :]
                        gi_rhs = oh_t[32 * m:32 * m + 8, t % 4, :]
                        tp = (32 * m, 0)
                    else:
                        gi_lhs = girz_sb[0:8, 1, :]
                        gi_rhs = oh_t[0:8, t % 4, :]
                        tp = (0, 0)
                    nc.tensor.matmul(tgts[m], lhsT=gi_lhs, rhs=gi_rhs,
                                     start=True, stop=False, tile_position=tp)
                for m in range(4):
                    for k in range(KC):
                        nc.tensor.matmul(
                            tgts[m], lhsT=whh_sb[:, k, m * 128:(m + 1) * 128],
                            rhs=h[:, k, :], start=False, stop=(k == KC - 1))
                    if m == 1:
                        r16 = g_pool.tile([128, 2, n_b], F16, tag=f"r{par}",
                                          name="r16")
                        nc.scalar.activation(r16, rz1, AF.Sigmoid)
                        # tanh(s-1) queues here - after sigma1, before sigma2:
                        # its input is ready by now (no ACT idle), and sigma2's
                        # consumer runs a slot later so it tolerates the delay
                        if mid_cb is not None:
                            mid_cb()
                    elif m == 3:
                        z16 = g_pool.tile([128, 2, n_b], F16, tag=f"z{par}",
                                          name="z16")
                        nc.scalar.activation(z16, rz2, AF.Sigmoid)  # = 1-z

                hn = ps_hn.tile([128, 2, n_b], F32, tag="hn", name="hn")
                for j in range(2):
                    for k in range(KC):
                        nc.tensor.matmul(
                            hn[:, j, :], lhsT=whh_sb[:, k, (4 + j) * 128:(5 + j) * 128],
                            rhs=h[:, k, :], start=(k == 0),
                            stop=(k == KC - 1) and not use_bhh_n)
                    if use_bhh_n:
                        nc.tensor.matmul(
                            hn[:, j, :], lhsT=bhhn_sb[:, j * 128:(j + 1) * 128],
                            rhs=ones_row, start=False, stop=True)

                p16 = g_pool.tile([128, 2, n_b], F16, tag=f"p{par}", name="p16")
                nc.vector.tensor_mul(p16, r16, hn)
                npre = g_pool.tile([128, 2, n_b], F16, tag=f"np{par}", name="npre")
                nc.vector.tensor_add(npre, p16, st["iq"][t // 2][:, t % 2])
                st["r16"], st["z16"], st["npre"] = r16, z16, npre

            def ph2(st, t, par):
                n16 = g_pool.tile([128, 2, n_b], F16, tag=f"n{par}", name="n16")
                nc.scalar.activation(n16, st["npre"], AF.Tanh)
                d16 = g_pool.tile([128, 2, n_b], F16, tag=f"d{par}", name="d16")
                nc.vector.tensor_tensor(d16, n16, st["h"], ALU.subtract)
                e16 = g_pool.tile([128, 2, n_b], F16, tag=f"e{par}", name="e16")
                nc.gpsimd.tensor_mul(e16, st["z16"], d16)
                st["n16"], st["d16"], st["e16"] = n16, d16, e16

            def ph3a(st, par):
                # h update only - emitted at slot start so DVE produces h'
                # before the PE needs it (logits + next-step recurrence).
                h_new = h_pool.tile([128, KC, n_b], F16, tag=f"h{par}",
                                    name="h_new")
                nc.vector.tensor_add(h_new, st["h"], st["e16"])
                st["h"] = h_new
                return h_new

            def ph3b(h_new, t, par, slot_i, lgctx):
                # logits matmul - deferred one further slot so the d->e->h'
                # cross-engine chain has a full slot of slack before the PE
                # needs h_new
                if slot_i % 4 == 0:
                    lgctx["lg4"] = ps_lg.tile([128, n_b], F32, tag="lg",
                                              name="lg4")
                row = 32 * (slot_i % 4)
                lgt = lgctx["lg4"][row:row + 32, :]
                for k in range(KC):
                    nc.tensor.matmul(
                        lgt, lhsT=wout_sb[:, k, :], rhs=h_new[:, k, :],
                        start=(k == 0), stop=(k == KC - 1) and not use_bout,
                        tile_position=(0, row))
                if use_bout:
                    nc.tensor.matmul(lgt, lhsT=bout_sb, rhs=ones_row,
                                     start=False, stop=True, tile_position=(0, row))
                if slot_i % 4 == 3:
                    lg_sb = lg_pool.tile([128, n_b], F16, tag="lg_sb",
                                         name="lg_sb")
                    nc.scalar.copy(lg_sb, lgctx["lg4"])
                    nc.sync.dma_start(outT[slot_i // 4], lg_sb)

            # ---- software-pipelined emission ------------------------------
            # ph1(s) | ph2 at s+1 | ph3 at s+2; retire depth min(P,2) keeps
            # the same-parity h-update (ph3) ahead of its next ph1 for any P.
            lgctx = {}
            pending = []  # [state, t, par, slot_i, ph2_done]
            pend3 = []  # [h_new, t, par, slot_i] awaiting deferred ph3b
            depth = min(P, 2)

            def retire2():
                it = pending.pop(0)
                if not it[4]:
                    ph2(it[0], it[1], it[2])
                h_new = ph3a(it[0], it[2])
                pend3.append([h_new, it[1], it[2], it[3]])

            def retire3():
                i3 = pend3.pop(0)
                ph3b(i3[0], i3[1], i3[2], i3[3], lgctx)

            slot_i = 0
            for base in range(0, n_chunks, P):
                states = [chunk_prologue(base + par, par) for par in range(P)]
                for t in range(L):
                    for par in range(P):
                        if len(pending) >= depth:
                            retire2()

                        def mid_cb():
                            if pending and not pending[-1][4]:
                                ph2(pending[-1][0], pending[-1][1],
                                    pending[-1][2])
                                pending[-1][4] = True

                        ph1(states[par], t, par, mid_cb=mid_cb)
                        if len(pend3) >= 2:
                            retire3()
                        if pending and not pending[-1][4]:
                            ph2(pending[-1][0], pending[-1][1], pending[-1][2])
                            pending[-1][4] = True
                        pending.append([states[par], t, par, slot_i, False])
                        slot_i += 1
            while pending:
                retire2()
            while pend3:
                retire3()

    nc.compile()
    return nc


def make_in_maps(latent_context, target_sequence, emb_table, W_ih, W_hh,
                 b_ih, b_hh, W_out, b_out, b_core=B_CORE, n_b=N_B, mm=None):
    """Shard + lay out the inputs for each core (host-side index/cast work)."""
    lat = np.asarray(latent_context, dtype=np.float32)
    tok = np.asarray(target_sequence).astype(np.int64)
    emb = np.asarray(emb_table, dtype=np.float32)
    W_ih = np.asarray(W_ih, dtype=np.float32)
    W_hh = np.asarray(W_hh, dtype=np.float32)
    b_ih = np.asarray(b_ih, dtype=np.float32)
    b_hh = np.asarray(b_hh, dtype=np.float32)
    W_out = np.asarray(W_out, dtype=np.float32)
    b_out = np.asarray(b_out, dtype=np.float32)

    n_chunks = b_core // n_b
    btot = lat.shape[0]

    # sign flip for the z gate rows (256:512) so sigmoid gives 1-z
    sign = np.ones((G,), np.float32)
    sign[256:512] = -1.0

    whhT = (W_hh * sign[:, None]).T.reshape(KC, 128, G).transpose(1, 0, 2)
    whhT = np.ascontiguousarray(whhT.astype(np.float16))
    W_out_pad = np.zeros((32, LATENT), np.float32)
    W_out_pad[:NTOK] = W_out
    woutT = np.ascontiguousarray(
        W_out_pad.T.reshape(KC, 128, 32).transpose(1, 0, 2).astype(np.float16))

    tok_in = np.concatenate(
        [np.full((btot, 1), START, tok.dtype), tok[:, :L - 1]], axis=1)  # (B, L)

    gi_tbl = emb @ W_ih.T + b_ih  # (V, G)
    gi_rz = gi_tbl[:, 0:512] * sign[None, 0:512]
    b_hh_rz = b_hh[0:512] * sign[0:512]
    girz = np.zeros((128, 2, 128), np.float32)
    for m in range(4):
        s, row0 = (0, 32 * m) if m < 3 else (1, 0)
        girz[row0:row0 + V, s, :] = gi_rz[:, m * 128:(m + 1) * 128]
        girz[row0 + 6, s, :] = b_hh_rz[m * 128:(m + 1) * 128]
    girz = np.ascontiguousarray(girz.astype(np.float16))

    i_n_tbl = gi_tbl[:, 512:G].astype(np.float16)  # (V, 256)

    use_bhh_n = bool(np.any(b_hh[512:]))
    use_bout = bool(np.any(b_out))

    n_cores_eff = btot // b_core
    in_maps = []
    for i in range(n_cores_eff):
        sl = slice(i * b_core, (i + 1) * b_core)
        lat_i = lat[sl]
        tok_i = tok_in[sl]

        latT = lat_i.reshape(n_chunks, n_b, KC, 128).transpose(0, 3, 2, 1)
        latT = np.ascontiguousarray(latT.astype(np.float16))

        tj = tok_i.reshape(n_chunks, n_b, L).transpose(0, 2, 1)  # (c, t, j)
        ohc = np.zeros((n_chunks, 8, L, n_b), np.float16)
        for v in range(V):
            ohc[:, v] = (tj == v)
        ohc[:, 6] = 1.0
        ohc = np.ascontiguousarray(ohc)

        g = i_n_tbl[tok_i]  # (b_core, L, 256) f16
        g = g.reshape(n_chunks, n_b, L, KC, 128).transpose(0, 4, 2, 3, 1)
        in16 = np.ascontiguousarray(g)

        m = {
            "latT": latT,
            "oh": ohc,
            "in16": in16,
            "whhT": whhT,
            "woutT": woutT,
            "girz": girz,
        }
        if use_bhh_n:
            m["bhhn"] = np.ascontiguousarray(
                b_hh[512:].reshape(1, LATENT).astype(np.float16))
        if use_bout:
            b_out_pad = np.zeros((1, 32), np.float32)
            b_out_pad[0, :NTOK] = b_out
            m["bout"] = np.ascontiguousarray(b_out_pad.astype(np.float16))
        in_maps.append(m)
    return in_maps


def unpack_out(o, b_core=B_CORE, n_b=N_B):
    """outT (n_packs, 128, n_b) -> logits (b_core, L, NTOK)."""
    o = np.asarray(o)
    n_chunks = b_core // n_b
    P = min(P_MAX, n_chunks)
    out = np.empty((b_core, L, NTOK), np.float32)
    i = 0
    for base in range(0, n_chunks, P):
        for t in range(L):
            for par in range(P):
                c = base + par
                out[c * n_b:(c + 1) * n_b, t, :] = \
                    o[i // 4, 32 * (i % 4):32 * (i % 4) + NTOK, :].T
                i += 1
    return out


def run(inputs, trace=False, b_core=B_CORE, mm=None):
    in_maps = make_in_maps(b_core=b_core, **inputs)
    use_bhh_n = "bhhn" in in_maps[0]
    use_bout = "bout" in in_maps[0]
    nc = _get_program(b_core, use_bhh_n, use_bout)
    core_ids = list(range(len(in_maps)))
    res = bass_utils.run_bass_kernel_spmd(nc, in_maps, core_ids, trace=trace)
    outs = [unpack_out(res.results[i]["outT"], b_core=b_core) for i in core_ids]
    return np.concatenate(outs, axis=0), res


_PROGRAM_CACHE = {}


def _get_program(b_core, use_bhh_n, use_bout):
    key = (b_core, use_bhh_n, use_bout)
    if key not in _PROGRAM_CACHE:
        _PROGRAM_CACHE[key] = build_program(
            b_core=b_core, use_bhh_n=use_bhh_n, use_bout=use_bout)
    return _PROGRAM_CACHE[key]


def kernel(**inputs) -> np.ndarray:
    out, _ = run(inputs, trace=False)
    return out


# revision 31
# speedup vs baseline: 1.0821x; 1.0821x over previous
"""Trainium2 Bass kernel for nn_AutoregressiveRoutingHead (v3).

Model (per batch row b):
    tok_in = [START, tgt[0..6]]                       # teacher forcing, START=5
    x_t    = emb[tok_in[t]]                           # (HID,)
    gi     = x_t @ W_ih.T + b_ih                      # (768,)
    gh     = h @ W_hh.T + b_hh                        # (768,)
    r = sigmoid(gi_r + gh_r); z = sigmoid(gi_z + gh_z)
    n = tanh(gi_n + r * gh_n)
    h' = (1-z)*n + z*h = h + (1-z)*(n - h)
    logits_t = h' @ W_out.T + b_out                   # (5,)

v3 strategy (pure data parallel over batch, 65536 -> 8 x 8192; per core
8192 -> 16 column chunks of 512, processed P=4 at a time as a software
pipeline):

- Host precomputes the token one-hot (incl START at t=0), the transposed f16
  initial hidden state, and gathered n-gate inputs
  in16[b,t] = emb[tok_in[b,t]] @ W_ih_n.T + b_ih_n.
- z-gate weights/tables are NEGATED on the host so sigmoid directly yields
  z' = 1 - z, giving h' = h + z'*(n - h).
- b_hh (r/z part) rides row 6 of the one-hot (always 1.0) through the K=8
  gather matmul; no activation biases needed anywhere.
- Each chunk-step is emitted in 3 phases across pipeline slots so no engine
  queue head-of-line blocks on the serial GRU chain:
    ph1(s):  rz matmuls + 2 sigmoids + hn matmuls + p = r*gh_n + npre = p+i_n
    ph2(s+1): tanh + d = n-h + e = z'*d (GpSimd)
    ph3(s+2): h' = h+e + logits matmul (+ PSUM->SBUF copy / DMA every 4 slots)
  With P=4 interleaved chunks the ~10us chain hides under the ~3.5us/slot
  engine throughput bound.
- Logits: W_out is zero-padded to 32 rows; 4 consecutive slots write one
  PSUM bank at column groups 0/32/64/96, drained by one DVE copy + one DMA
  per 4 slots.
- PSUM: rz pool 2x2 banks + hn 1x2 banks + lg 2x1 bank = 8 banks.
"""

import numpy as np

import concourse.bass as bass
import concourse.mybir as mybir
import concourse.tile as tile
from concourse import bacc, bass_utils

F32 = mybir.dt.float32
F16 = mybir.dt.float16
AF = mybir.ActivationFunctionType
ALU = mybir.AluOpType

N_CORES = 8
B = 65536
L = 8
LATENT = 256
HID = 128
NTOK = 5
V = NTOK + 1  # vocab incl <start>
START = NTOK
G = 3 * LATENT  # 768 gate rows
KC = LATENT // 128  # 2 contraction chunks

B_CORE = B // N_CORES
N_B = 512
P_MAX = 4  # chunks interleaved in the software pipeline


def build_program(b_core=B_CORE, n_b=N_B, use_bhh_n=False, use_bout=False):
    """Build + compile the per-core Bass program (SPMD: same program, 8 cores)."""
    nc = bacc.Bacc("TRN2", target_bir_lowering=False, debug=False)
    n_chunks = b_core // n_b
    P = min(P_MAX, n_chunks)
    assert n_chunks % P == 0
    n_items = n_chunks * L
    assert n_items % 4 == 0
    n_packs = n_items // 4

    # ---- DRAM I/O ----------------------------------------------------------
    latT = nc.dram_tensor("latT", [n_chunks, 128, KC, n_b], F16, kind="ExternalInput").ap()
    oh = nc.dram_tensor("oh", [n_chunks, 8, L, n_b], F16, kind="ExternalInput").ap()
    in16 = nc.dram_tensor("in16", [n_chunks, 128, L, KC, n_b], F16, kind="ExternalInput").ap()
    whhT = nc.dram_tensor("whhT", [128, KC, G], F16, kind="ExternalInput").ap()
    woutT = nc.dram_tensor("woutT", [128, KC, 32], F16, kind="ExternalInput").ap()
    girz = nc.dram_tensor("girz", [128, 2, 128], F16, kind="ExternalInput").ap()
    bhhn = bout = None
    if use_bhh_n:
        bhhn = nc.dram_tensor("bhhn", [1, LATENT], F16, kind="ExternalInput").ap()
    if use_bout:
        bout = nc.dram_tensor("bout", [1, 32], F16, kind="ExternalInput").ap()
    # logits: pipeline slot i -> pack i//4, rows 32*(i%4)+v
    outT = nc.dram_tensor("outT", [n_packs, 128, n_b], F16, kind="ExternalOutput").ap()

    with tile.TileContext(nc) as tc:
        with tc.tile_pool(name="singles", bufs=1) as singles, \
             tc.tile_pool(name="hpool", bufs=4) as h_pool, \
             tc.tile_pool(name="ohpool", bufs=1) as oh_pool, \
             tc.tile_pool(name="inpool", bufs=2) as in_pool, \
             tc.tile_pool(name="gates", bufs=1) as g_pool, \
             tc.tile_pool(name="lgpool", bufs=2) as lg_pool, \
             tc.tile_pool(name="ps_rz", bufs=2, space="PSUM") as ps_rz, \
             tc.tile_pool(name="ps_hn", bufs=1, space="PSUM") as ps_hn, \
             tc.tile_pool(name="ps_lg", bufs=2, space="PSUM") as ps_lg:

            # ---- constants / weights in SBUF -------------------------------
            whh_sb = singles.tile([128, KC, G], F16, tag="whh")
            nc.sync.dma_start(whh_sb, whhT)
            wout_sb = singles.tile([128, KC, 32], F16, tag="wout")
            nc.sync.dma_start(wout_sb, woutT)
            girz_sb = singles.tile([128, 2, 128], F16, tag="girz")
            nc.sync.dma_start(girz_sb, girz)
            bhhn_sb = bout_sb = ones_row = None
            if use_bhh_n or use_bout:
                ones_row = singles.tile([1, n_b], F16, tag="ones_row")
                nc.vector.memset(ones_row, 1.0)
            if use_bhh_n:
                bhhn_sb = singles.tile([1, LATENT], F16, tag="bhhn")
                nc.sync.dma_start(bhhn_sb, bhhn)
            if use_bout:
                bout_sb = singles.tile([1, 32], F16, tag="bout")
                nc.sync.dma_start(bout_sb, bout)

            def chunk_prologue(c, par):
                h = h_pool.tile([128, KC, n_b], F16, tag=f"h{par}", name="h0")
                nc.sync.dma_start(h, latT[c])
                # oh/in16 split into 4 quarter-tiles (2 steps each) for finer
                # SBUF ring recycling (cross-group prefetch) + DMA spreading
                ohq = []
                for q in range(2):
                    oq = oh_pool.tile([128, 4, n_b], F16, tag=f"oh{par}",
                                      name=f"oh_h{q}")
                    for g in range(3):
                        nc.sync.dma_start(oq[32 * g:32 * g + 8],
                                          oh[c, :, 4 * q:4 * q + 4])
                    ohq.append(oq)
                iq = []
                for q in range(4):
                    tq = in_pool.tile([128, 2, KC, n_b], F16, tag=f"in{par}",
                                      name=f"in_q{q}")
                    nc.sync.dma_start(tq, in16[c, :, 2 * q:2 * q + 2])
                    iq.append(tq)
                return {"ohq": ohq, "iq": iq, "h": h}

            # ---- pipeline phases ------------------------------------------
            def ph1(st, t, par):
                # All 4 K=8 gather matmuls first (row groups 0/32/64 let the
                # pairs co-stream), then the K=128 recurrence chains (same-
                # group back-to-back matmuls hide their LDWEIGHTS).
                h = st["h"]
                oh_t = st["ohq"][t // 4]
                rz1 = ps_rz.tile([128, 2, n_b], F32, tag="rz", name="rz1")
                rz2 = ps_rz.tile([128, 2, n_b], F32, tag="rz", name="rz2")
                tgts = [rz1[:, 0, :], rz1[:, 1, :], rz2[:, 0, :], rz2[:, 1, :]]
                for m in range(4):
                    if m < 3:
                        gi_lhs = girz_sb[32 * m:32 * m + 8, 0, :]
                        gi_rhs = oh_t[32 * m:32 * m + 8, t % 4, :]
                        tp = (32 * m, 0)
                    else:
                        gi_lhs = girz_sb[0:8, 1, :]
                        gi_rhs = oh_t[0:8, t % 4, :]
                        tp = (0, 0)
                    nc.tensor.matmul(tgts[m], lhsT=gi_lhs, rhs=gi_rhs,
                                     start=True, stop=False, tile_position=tp)
                for m in range(4):
                    for k in range(KC):
                        nc.tensor.matmul(
                            tgts[m], lhsT=whh_sb[:, k, m * 128:(m + 1) * 128],
                            rhs=h[:, k, :], start=False, stop=(k == KC - 1))
                    if m == 1:
                        r16 = g_pool.tile([128, 2, n_b], F16, tag=f"r{par}",
                                          name="r16")
                        nc.scalar.activation(r16, rz1, AF.Sigmoid)
                    elif m == 3:
                        z16 = g_pool.tile([128, 2, n_b], F16, tag=f"z{par}",
                                          name="z16")
                        nc.scalar.activation(z16, rz2, AF.Sigmoid)  # = 1-z

                hn = ps_hn.tile([128, 2, n_b], F32, tag="hn", name="hn")
                for j in range(2):
                    for k in range(KC):
                        nc.tensor.matmul(
                            hn[:, j, :], lhsT=whh_sb[:, k, (4 + j) * 128:(5 + j) * 128],
                            rhs=h[:, k, :], start=(k == 0),
                            stop=(k == KC - 1) and not use_bhh_n)
                    if use_bhh_n:
                        nc.tensor.matmul(
                            hn[:, j, :], lhsT=bhhn_sb[:, j * 128:(j + 1) * 128],
                            rhs=ones_row, start=False, stop=True)

                p16 = g_pool.tile([128, 2, n_b], F16, tag=f"p{par}", name="p16")
                nc.vector.tensor_mul(p16, r16, hn)
                npre = g_pool.tile([128, 2, n_b], F16, tag=f"np{par}", name="npre")
                nc.vector.tensor_add(npre, p16, st["iq"][t // 2][:, t % 2])
                st["r16"], st["z16"], st["npre"] = r16, z16, npre

            def ph2(st, t, par):
                n16 = g_pool.tile([128, 2, n_b], F16, tag=f"n{par}", name="n16")
                nc.scalar.activation(n16, st["npre"], AF.Tanh)
                d16 = g_pool.tile([128, 2, n_b], F16, tag=f"d{par}", name="d16")
                nc.vector.tensor_tensor(d16, n16, st["h"], ALU.subtract)
                e16 = g_pool.tile([128, 2, n_b], F16, tag=f"e{par}", name="e16")
                nc.gpsimd.tensor_mul(e16, st["z16"], d16)
                st["n16"], st["d16"], st["e16"] = n16, d16, e16

            def ph3a(st, par):
                # h update only - emitted at slot start so DVE produces h'
                # before the PE needs it (logits + next-step recurrence).
                h_new = h_pool.tile([128, KC, n_b], F16, tag=f"h{par}",
                                    name="h_new")
                nc.vector.tensor_add(h_new, st["h"], st["e16"])
                st["h"] = h_new
                return h_new

            def ph3b(h_new, t, par, slot_i, lgctx):
                # logits matmul - deferred one further slot so the d->e->h'
                # cross-engine chain has a full slot of slack before the PE
                # needs h_new
                if slot_i % 4 == 0:
                    lgctx["lg4"] = ps_lg.tile([128, n_b], F32, tag="lg",
                                              name="lg4")
                row = 32 * (slot_i % 4)
                lgt = lgctx["lg4"][row:row + 32, :]
                for k in range(KC):
                    nc.tensor.matmul(
                        lgt, lhsT=wout_sb[:, k, :], rhs=h_new[:, k, :],
                        start=(k == 0), stop=(k == KC - 1) and not use_bout,
                        tile_position=(0, row))
                if use_bout:
                    nc.tensor.matmul(lgt, lhsT=bout_sb, rhs=ones_row,
                                     start=False, stop=True, tile_position=(0, row))
                if slot_i % 4 == 3:
                    lg_sb = lg_pool.tile([128, n_b], F16, tag="lg_sb",
                                         name="lg_sb")
                    nc.scalar.copy(lg_sb, lgctx["lg4"])
                    nc.sync.dma_start(outT[slot_i // 4], lg_sb)

            # ---- software-pipelined emission ------------------------------
            # ph1(s) | ph2 at s+1 | ph3 at s+2; retire depth min(P,2) keeps
            # the same-parity h-update (ph3) ahead of its next ph1 for any P.
            lgctx = {}
            pending = []  # [state, t, par, slot_i, ph2_done]
            pend3 = []  # [h_new, t, par, slot_i] awaiting deferred ph3b
            depth = min(P, 2)

            def retire2(split=False):
                it = pending.pop(0)
                if not it[4]:
                    ph2(it[0], it[1], it[2])
                if split:
                    return it
                h_new = ph3a(it[0], it[2])
                pend3.append([h_new, it[1], it[2], it[3]])
                return None

            def retire3():
                i3 = pend3.pop(0)
                ph3b(i3[0], i3[1], i3[2], i3[3], lgctx)

            slot_i = 0
            for base in range(0, n_chunks, P):
                states = [chunk_prologue(base + par, par) for par in range(P)]
                for t in range(L):
                    for par in range(P):
                        # For P>=3, defer the h-update (ph3a) until after ph1
                        # so p/npre lead the DVE queue instead of queuing
                        # behind h'(s-2), which waits on the slow GpSimd e op
                        # (the measured 2-slot pacing cycle). Same-parity
                        # ordering is safe: its next ph1 is P slots away.
                        it2 = None
                        if len(pending) >= depth:
                            it2 = retire2(split=(P >= 3))
                        ph1(states[par], t, par)
                        if it2 is not None:
                            h_new = ph3a(it2[0], it2[2])
                            pend3.append([h_new, it2[1], it2[2], it2[3]])
                        if len(pend3) >= 2:
                            retire3()
                        if pending and not pending[-1][4]:
                            ph2(pending[-1][0], pending[-1][1], pending[-1][2])
                            pending[-1][4] = True
                        pending.append([states[par], t, par, slot_i, False])
                        slot_i += 1
            while pending:
                retire2()
            while pend3:
                retire3()

    nc.compile()
    return nc


def make_in_maps(latent_context, target_sequence, emb_table, W_ih, W_hh,
                 b_ih, b_hh, W_out, b_out, b_core=B_CORE, n_b=N_B, mm=None):
    """Shard + lay out the inputs for each core (host-side index/cast work)."""
    lat = np.asarray(latent_context, dtype=np.float32)
    tok = np.asarray(target_sequence).astype(np.int64)
    emb = np.asarray(emb_table, dtype=np.float32)
    W_ih = np.asarray(W_ih, dtype=np.float32)
    W_hh = np.asarray(W_hh, dtype=np.float32)
    b_ih = np.asarray(b_ih, dtype=np.float32)
    b_hh = np.asarray(b_hh, dtype=np.float32)
    W_out = np.asarray(W_out, dtype=np.float32)
    b_out = np.asarray(b_out, dtype=np.float32)

    n_chunks = b_core // n_b
    btot = lat.shape[0]

    # sign flip for the z gate rows (256:512) so sigmoid gives 1-z
    sign = np.ones((G,), np.float32)
    sign[256:512] = -1.0

    whhT = (W_hh * sign[:, None]).T.reshape(KC, 128, G).transpose(1, 0, 2)
    whhT = np.ascontiguousarray(whhT.astype(np.float16))
    W_out_pad = np.zeros((32, LATENT), np.float32)
    W_out_pad[:NTOK] = W_out
    woutT = np.ascontiguousarray(
        W_out_pad.T.reshape(KC, 128, 32).transpose(1, 0, 2).astype(np.float16))

    tok_in = np.concatenate(
        [np.full((btot, 1), START, tok.dtype), tok[:, :L - 1]], axis=1)  # (B, L)

    gi_tbl = emb @ W_ih.T + b_ih  # (V, G)
    gi_rz = gi_tbl[:, 0:512] * sign[None, 0:512]
    b_hh_rz = b_hh[0:512] * sign[0:512]
    girz = np.zeros((128, 2, 128), np.float32)
    for m in range(4):
        s, row0 = (0, 32 * m) if m < 3 else (1, 0)
        girz[row0:row0 + V, s, :] = gi_rz[:, m * 128:(m + 1) * 128]
        girz[row0 + 6, s, :] = b_hh_rz[m * 128:(m + 1) * 128]
    girz = np.ascontiguousarray(girz.astype(np.float16))

    i_n_tbl = gi_tbl[:, 512:G].astype(np.float16)  # (V, 256)

    use_bhh_n = bool(np.any(b_hh[512:]))
    use_bout = bool(np.any(b_out))

    n_cores_eff = btot // b_core
    in_maps = []
    for i in range(n_cores_eff):
        sl = slice(i * b_core, (i + 1) * b_core)
        lat_i = lat[sl]
        tok_i = tok_in[sl]

        latT = lat_i.reshape(n_chunks, n_b, KC, 128).transpose(0, 3, 2, 1)
        latT = np.ascontiguousarray(latT.astype(np.float16))

        tj = tok_i.reshape(n_chunks, n_b, L).transpose(0, 2, 1)  # (c, t, j)
        ohc = np.zeros((n_chunks, 8, L, n_b), np.float16)
        for v in range(V):
            ohc[:, v] = (tj == v)
        ohc[:, 6] = 1.0
        ohc = np.ascontiguousarray(ohc)

        g = i_n_tbl[tok_i]  # (b_core, L, 256) f16
        g = g.reshape(n_chunks, n_b, L, KC, 128).transpose(0, 4, 2, 3, 1)
        in16 = np.ascontiguousarray(g)

        m = {
            "latT": latT,
            "oh": ohc,
            "in16": in16,
            "whhT": whhT,
            "woutT": woutT,
            "girz": girz,
        }
        if use_bhh_n:
            m["bhhn"] = np.ascontiguousarray(
                b_hh[512:].reshape(1, LATENT).astype(np.float16))
        if use_bout:
            b_out_pad = np.zeros((1, 32), np.float32)
            b_out_pad[0, :NTOK] = b_out
            m["bout"] = np.ascontiguousarray(b_out_pad.astype(np.float16))
        in_maps.append(m)
    return in_maps


def unpack_out(o, b_core=B_CORE, n_b=N_B):
    """outT (n_packs, 128, n_b) -> logits (b_core, L, NTOK)."""
    o = np.asarray(o)
    n_chunks = b_core // n_b
    P = min(P_MAX, n_chunks)
    out = np.empty((b_core, L, NTOK), np.float32)
    i = 0
    for base in range(0, n_chunks, P):
        for t in range(L):
            for par in range(P):
                c = base + par
                out[c * n_b:(c + 1) * n_b, t, :] = \
                    o[i // 4, 32 * (i % 4):32 * (i % 4) + NTOK, :].T
                i += 1
    return out


def run(inputs, trace=False, b_core=B_CORE, mm=None):
    in_maps = make_in_maps(b_core=b_core, **inputs)
    use_bhh_n = "bhhn" in in_maps[0]
    use_bout = "bout" in in_maps[0]
    nc = _get_program(b_core, use_bhh_n, use_bout)
    core_ids = list(range(len(in_maps)))
    res = bass_utils.run_bass_kernel_spmd(nc, in_maps, core_ids, trace=trace)
    outs = [unpack_out(res.results[i]["outT"], b_core=b_core) for i in core_ids]
    return np.concatenate(outs, axis=0), res


_PROGRAM_CACHE = {}


def _get_program(b_core, use_bhh_n, use_bout):
    key = (b_core, use_bhh_n, use_bout)
    if key not in _PROGRAM_CACHE:
        _PROGRAM_CACHE[key] = build_program(
            b_core=b_core, use_bhh_n=use_bhh_n, use_bout=use_bout)
    return _PROGRAM_CACHE[key]


def kernel(**inputs) -> np.ndarray:
    out, _ = run(inputs, trace=False)
    return out


# revision 32
# speedup vs baseline: 1.0931x; 1.0102x over previous
"""Trainium2 Bass kernel for nn_AutoregressiveRoutingHead (v3).

Model (per batch row b):
    tok_in = [START, tgt[0..6]]                       # teacher forcing, START=5
    x_t    = emb[tok_in[t]]                           # (HID,)
    gi     = x_t @ W_ih.T + b_ih                      # (768,)
    gh     = h @ W_hh.T + b_hh                        # (768,)
    r = sigmoid(gi_r + gh_r); z = sigmoid(gi_z + gh_z)
    n = tanh(gi_n + r * gh_n)
    h' = (1-z)*n + z*h = h + (1-z)*(n - h)
    logits_t = h' @ W_out.T + b_out                   # (5,)

v3 strategy (pure data parallel over batch, 65536 -> 8 x 8192; per core
8192 -> 16 column chunks of 512, processed P=4 at a time as a software
pipeline):

- Host precomputes the token one-hot (incl START at t=0), the transposed f16
  initial hidden state, and gathered n-gate inputs
  in16[b,t] = emb[tok_in[b,t]] @ W_ih_n.T + b_ih_n.
- z-gate weights/tables are NEGATED on the host so sigmoid directly yields
  z' = 1 - z, giving h' = h + z'*(n - h).
- b_hh (r/z part) rides row 6 of the one-hot (always 1.0) through the K=8
  gather matmul; no activation biases needed anywhere.
- Each chunk-step is emitted in 3 phases across pipeline slots so no engine
  queue head-of-line blocks on the serial GRU chain:
    ph1(s):  rz matmuls + 2 sigmoids + hn matmuls + p = r*gh_n + npre = p+i_n
    ph2(s+1): tanh + d = n-h + e = z'*d (GpSimd)
    ph3(s+2): h' = h+e + logits matmul (+ PSUM->SBUF copy / DMA every 4 slots)
  With P=4 interleaved chunks the ~10us chain hides under the ~3.5us/slot
  engine throughput bound.
- Logits: W_out is zero-padded to 32 rows; 4 consecutive slots write one
  PSUM bank at column groups 0/32/64/96, drained by one DVE copy + one DMA
  per 4 slots.
- PSUM: rz pool 2x2 banks + hn 1x2 banks + lg 2x1 bank = 8 banks.
"""

import numpy as np

import concourse.bass as bass
import concourse.mybir as mybir
import concourse.tile as tile
from concourse import bacc, bass_utils

F32 = mybir.dt.float32
F16 = mybir.dt.float16
AF = mybir.ActivationFunctionType
ALU = mybir.AluOpType

N_CORES = 8
B = 65536
L = 8
LATENT = 256
HID = 128
NTOK = 5
V = NTOK + 1  # vocab incl <start>
START = NTOK
G = 3 * LATENT  # 768 gate rows
KC = LATENT // 128  # 2 contraction chunks

B_CORE = B // N_CORES
N_B = 512
P_MAX = 4  # chunks interleaved in the software pipeline


def build_program(b_core=B_CORE, n_b=N_B, use_bhh_n=False, use_bout=False):
    """Build + compile the per-core Bass program (SPMD: same program, 8 cores)."""
    nc = bacc.Bacc("TRN2", target_bir_lowering=False, debug=False)
    n_chunks = b_core // n_b
    P = min(P_MAX, n_chunks)
    assert n_chunks % P == 0
    n_items = n_chunks * L
    assert n_items % 4 == 0
    n_packs = n_items // 4

    # ---- DRAM I/O ----------------------------------------------------------
    latT = nc.dram_tensor("latT", [n_chunks, 128, KC, n_b], F16, kind="ExternalInput").ap()
    oh = nc.dram_tensor("oh", [n_chunks, 8, L, n_b], F16, kind="ExternalInput").ap()
    in16 = nc.dram_tensor("in16", [n_chunks, 128, L, KC, n_b], F16, kind="ExternalInput").ap()
    whhT = nc.dram_tensor("whhT", [128, KC, G], F16, kind="ExternalInput").ap()
    woutT = nc.dram_tensor("woutT", [128, KC, 32], F16, kind="ExternalInput").ap()
    girz = nc.dram_tensor("girz", [128, 2, 128], F16, kind="ExternalInput").ap()
    bhhn = bout = None
    if use_bhh_n:
        bhhn = nc.dram_tensor("bhhn", [1, LATENT], F16, kind="ExternalInput").ap()
    if use_bout:
        bout = nc.dram_tensor("bout", [1, 32], F16, kind="ExternalInput").ap()
    # logits: pipeline slot i -> pack i//4, rows 32*(i%4)+v
    outT = nc.dram_tensor("outT", [n_packs, 128, n_b], F16, kind="ExternalOutput").ap()

    with tile.TileContext(nc) as tc:
        with tc.tile_pool(name="singles", bufs=1) as singles, \
             tc.tile_pool(name="hpool", bufs=4) as h_pool, \
             tc.tile_pool(name="ohpool", bufs=1) as oh_pool, \
             tc.tile_pool(name="inpool", bufs=2) as in_pool, \
             tc.tile_pool(name="gates", bufs=1) as g_pool, \
             tc.tile_pool(name="lgpool", bufs=2) as lg_pool, \
             tc.tile_pool(name="ps_rz", bufs=2, space="PSUM") as ps_rz, \
             tc.tile_pool(name="ps_hn", bufs=1, space="PSUM") as ps_hn, \
             tc.tile_pool(name="ps_lg", bufs=2, space="PSUM") as ps_lg:

            # ---- constants / weights in SBUF -------------------------------
            whh_sb = singles.tile([128, KC, G], F16, tag="whh")
            nc.sync.dma_start(whh_sb, whhT)
            wout_sb = singles.tile([128, KC, 32], F16, tag="wout")
            nc.sync.dma_start(wout_sb, woutT)
            girz_sb = singles.tile([128, 2, 128], F16, tag="girz")
            nc.sync.dma_start(girz_sb, girz)
            bhhn_sb = bout_sb = ones_row = None
            if use_bhh_n or use_bout:
                ones_row = singles.tile([1, n_b], F16, tag="ones_row")
                nc.vector.memset(ones_row, 1.0)
            if use_bhh_n:
                bhhn_sb = singles.tile([1, LATENT], F16, tag="bhhn")
                nc.sync.dma_start(bhhn_sb, bhhn)
            if use_bout:
                bout_sb = singles.tile([1, 32], F16, tag="bout")
                nc.sync.dma_start(bout_sb, bout)

            def chunk_prologue(c, par):
                h = h_pool.tile([128, KC, n_b], F16, tag=f"h{par}", name="h0")
                nc.sync.dma_start(h, latT[c])
                # oh/in16 split into 4 quarter-tiles (2 steps each) for finer
                # SBUF ring recycling (cross-group prefetch) + DMA spreading
                ohq = []
                for q in range(2):
                    oq = oh_pool.tile([128, 4, n_b], F16, tag=f"oh{par}",
                                      name=f"oh_h{q}")
                    for g in range(3):
                        nc.sync.dma_start(oq[32 * g:32 * g + 8],
                                          oh[c, :, 4 * q:4 * q + 4])
                    ohq.append(oq)
                iq = []
                for q in range(4):
                    tq = in_pool.tile([128, 2, KC, n_b], F16, tag=f"in{par}",
                                      name=f"in_q{q}")
                    nc.sync.dma_start(tq, in16[c, :, 2 * q:2 * q + 2])
                    iq.append(tq)
                return {"ohq": ohq, "iq": iq, "h": h}

            # ---- pipeline phases ------------------------------------------
            def ph1(st, t, par):
                # All 4 K=8 gather matmuls first (row groups 0/32/64 let the
                # pairs co-stream), then the K=128 recurrence chains (same-
                # group back-to-back matmuls hide their LDWEIGHTS).
                h = st["h"]
                oh_t = st["ohq"][t // 4]
                rz1 = ps_rz.tile([128, 2, n_b], F32, tag="rz", name="rz1")
                rz2 = ps_rz.tile([128, 2, n_b], F32, tag="rz", name="rz2")
                tgts = [rz1[:, 0, :], rz1[:, 1, :], rz2[:, 0, :], rz2[:, 1, :]]
                for m in range(4):
                    if m < 3:
                        gi_lhs = girz_sb[32 * m:32 * m + 8, 0, :]
                        gi_rhs = oh_t[32 * m:32 * m + 8, t % 4, :]
                        tp = (32 * m, 0)
                    else:
                        gi_lhs = girz_sb[0:8, 1, :]
                        gi_rhs = oh_t[0:8, t % 4, :]
                        tp = (0, 0)
                    nc.tensor.matmul(tgts[m], lhsT=gi_lhs, rhs=gi_rhs,
                                     start=True, stop=False, tile_position=tp)
                for m in range(4):
                    for k in range(KC):
                        nc.tensor.matmul(
                            tgts[m], lhsT=whh_sb[:, k, m * 128:(m + 1) * 128],
                            rhs=h[:, k, :], start=False, stop=(k == KC - 1))
                    if m == 1:
                        r16 = g_pool.tile([128, 2, n_b], F16, tag=f"r{par}",
                                          name="r16")
                        nc.scalar.activation(r16, rz1, AF.Sigmoid)
                    elif m == 3:
                        z16 = g_pool.tile([128, 2, n_b], F16, tag=f"z{par}",
                                          name="z16")
                        nc.scalar.activation(z16, rz2, AF.Sigmoid)  # = 1-z

                hn = ps_hn.tile([128, 2, n_b], F32, tag="hn", name="hn")
                for j in range(2):
                    for k in range(KC):
                        nc.tensor.matmul(
                            hn[:, j, :], lhsT=whh_sb[:, k, (4 + j) * 128:(5 + j) * 128],
                            rhs=h[:, k, :], start=(k == 0),
                            stop=(k == KC - 1) and not use_bhh_n)
                    if use_bhh_n:
                        nc.tensor.matmul(
                            hn[:, j, :], lhsT=bhhn_sb[:, j * 128:(j + 1) * 128],
                            rhs=ones_row, start=False, stop=True)

                p16 = g_pool.tile([128, 2, n_b], F16, tag=f"p{par}", name="p16")
                nc.vector.tensor_mul(p16, r16, hn)
                npre = g_pool.tile([128, 2, n_b], F16, tag=f"np{par}", name="npre")
                nc.vector.tensor_add(npre, p16, st["iq"][t // 2][:, t % 2])
                st["r16"], st["z16"], st["npre"] = r16, z16, npre

            def ph2(st, t, par):
                n16 = g_pool.tile([128, 2, n_b], F16, tag=f"n{par}", name="n16")
                nc.scalar.activation(n16, st["npre"], AF.Tanh)
                d16 = g_pool.tile([128, 2, n_b], F16, tag=f"d{par}", name="d16")
                nc.vector.tensor_tensor(d16, n16, st["h"], ALU.subtract)
                e16 = g_pool.tile([128, 2, n_b], F16, tag=f"e{par}", name="e16")
                nc.gpsimd.tensor_mul(e16, st["z16"], d16)
                st["n16"], st["d16"], st["e16"] = n16, d16, e16

            def ph3a(st, par):
                # h update only - emitted at slot start so DVE produces h'
                # before the PE needs it (logits + next-step recurrence).
                h_new = h_pool.tile([128, KC, n_b], F16, tag=f"h{par}",
                                    name="h_new")
                nc.vector.tensor_add(h_new, st["h"], st["e16"])
                st["h"] = h_new
                return h_new

            def ph3b(h_new, t, par, slot_i, lgctx):
                # logits matmul - deferred one further slot so the d->e->h'
                # cross-engine chain has a full slot of slack before the PE
                # needs h_new
                if slot_i % 4 == 0:
                    lgctx["lg4"] = ps_lg.tile([128, n_b], F32, tag="lg",
                                              name="lg4")
                row = 32 * (slot_i % 4)
                lgt = lgctx["lg4"][row:row + 32, :]
                for k in range(KC):
                    nc.tensor.matmul(
                        lgt, lhsT=wout_sb[:, k, :], rhs=h_new[:, k, :],
                        start=(k == 0), stop=(k == KC - 1) and not use_bout,
                        tile_position=(0, row))
                if use_bout:
                    nc.tensor.matmul(lgt, lhsT=bout_sb, rhs=ones_row,
                                     start=False, stop=True, tile_position=(0, row))
                if slot_i % 4 == 3:
                    lg_sb = lg_pool.tile([128, n_b], F16, tag="lg_sb",
                                         name="lg_sb")
                    nc.vector.tensor_copy(lg_sb, lgctx["lg4"])
                    nc.sync.dma_start(outT[slot_i // 4], lg_sb)

            # ---- software-pipelined emission ------------------------------
            # ph1(s) | ph2 at s+1 | ph3 at s+2; retire depth min(P,2) keeps
            # the same-parity h-update (ph3) ahead of its next ph1 for any P.
            lgctx = {}
            pending = []  # [state, t, par, slot_i, ph2_done]
            pend3 = []  # [h_new, t, par, slot_i] awaiting deferred ph3b
            depth = min(P, 2)

            def retire2(split=False):
                it = pending.pop(0)
                if not it[4]:
                    ph2(it[0], it[1], it[2])
                if split:
                    return it
                h_new = ph3a(it[0], it[2])
                pend3.append([h_new, it[1], it[2], it[3]])
                return None

            def retire3():
                i3 = pend3.pop(0)
                ph3b(i3[0], i3[1], i3[2], i3[3], lgctx)

            slot_i = 0
            for base in range(0, n_chunks, P):
                states = [chunk_prologue(base + par, par) for par in range(P)]
                for t in range(L):
                    for par in range(P):
                        # For P>=3, defer the h-update (ph3a) until after ph1
                        # so p/npre lead the DVE queue instead of queuing
                        # behind h'(s-2), which waits on the slow GpSimd e op
                        # (the measured 2-slot pacing cycle). Same-parity
                        # ordering is safe: its next ph1 is P slots away.
                        it2 = None
                        if len(pending) >= depth:
                            it2 = retire2(split=(P >= 3))
                        ph1(states[par], t, par)
                        if it2 is not None:
                            h_new = ph3a(it2[0], it2[2])
                            pend3.append([h_new, it2[1], it2[2], it2[3]])
                        if len(pend3) >= 2:
                            retire3()
                        if pending and not pending[-1][4]:
                            ph2(pending[-1][0], pending[-1][1], pending[-1][2])
                            pending[-1][4] = True
                        pending.append([states[par], t, par, slot_i, False])
                        slot_i += 1
            while pending:
                retire2()
            while pend3:
                retire3()

    nc.compile()
    return nc


def make_in_maps(latent_context, target_sequence, emb_table, W_ih, W_hh,
                 b_ih, b_hh, W_out, b_out, b_core=B_CORE, n_b=N_B, mm=None):
    """Shard + lay out the inputs for each core (host-side index/cast work)."""
    lat = np.asarray(latent_context, dtype=np.float32)
    tok = np.asarray(target_sequence).astype(np.int64)
    emb = np.asarray(emb_table, dtype=np.float32)
    W_ih = np.asarray(W_ih, dtype=np.float32)
    W_hh = np.asarray(W_hh, dtype=np.float32)
    b_ih = np.asarray(b_ih, dtype=np.float32)
    b_hh = np.asarray(b_hh, dtype=np.float32)
    W_out = np.asarray(W_out, dtype=np.float32)
    b_out = np.asarray(b_out, dtype=np.float32)

    n_chunks = b_core // n_b
    btot = lat.shape[0]

    # sign flip for the z gate rows (256:512) so sigmoid gives 1-z
    sign = np.ones((G,), np.float32)
    sign[256:512] = -1.0

    whhT = (W_hh * sign[:, None]).T.reshape(KC, 128, G).transpose(1, 0, 2)
    whhT = np.ascontiguousarray(whhT.astype(np.float16))
    W_out_pad = np.zeros((32, LATENT), np.float32)
    W_out_pad[:NTOK] = W_out
    woutT = np.ascontiguousarray(
        W_out_pad.T.reshape(KC, 128, 32).transpose(1, 0, 2).astype(np.float16))

    tok_in = np.concatenate(
        [np.full((btot, 1), START, tok.dtype), tok[:, :L - 1]], axis=1)  # (B, L)

    gi_tbl = emb @ W_ih.T + b_ih  # (V, G)
    gi_rz = gi_tbl[:, 0:512] * sign[None, 0:512]
    b_hh_rz = b_hh[0:512] * sign[0:512]
    girz = np.zeros((128, 2, 128), np.float32)
    for m in range(4):
        s, row0 = (0, 32 * m) if m < 3 else (1, 0)
        girz[row0:row0 + V, s, :] = gi_rz[:, m * 128:(m + 1) * 128]
        girz[row0 + 6, s, :] = b_hh_rz[m * 128:(m + 1) * 128]
    girz = np.ascontiguousarray(girz.astype(np.float16))

    i_n_tbl = gi_tbl[:, 512:G].astype(np.float16)  # (V, 256)

    use_bhh_n = bool(np.any(b_hh[512:]))
    use_bout = bool(np.any(b_out))

    n_cores_eff = btot // b_core
    in_maps = []
    for i in range(n_cores_eff):
        sl = slice(i * b_core, (i + 1) * b_core)
        lat_i = lat[sl]
        tok_i = tok_in[sl]

        latT = lat_i.reshape(n_chunks, n_b, KC, 128).transpose(0, 3, 2, 1)
        latT = np.ascontiguousarray(latT.astype(np.float16))

        tj = tok_i.reshape(n_chunks, n_b, L).transpose(0, 2, 1)  # (c, t, j)
        ohc = np.zeros((n_chunks, 8, L, n_b), np.float16)
        for v in range(V):
            ohc[:, v] = (tj == v)
        ohc[:, 6] = 1.0
        ohc = np.ascontiguousarray(ohc)

        g = i_n_tbl[tok_i]  # (b_core, L, 256) f16
        g = g.reshape(n_chunks, n_b, L, KC, 128).transpose(0, 4, 2, 3, 1)
        in16 = np.ascontiguousarray(g)

        m = {
            "latT": latT,
            "oh": ohc,
            "in16": in16,
            "whhT": whhT,
            "woutT": woutT,
            "girz": girz,
        }
        if use_bhh_n:
            m["bhhn"] = np.ascontiguousarray(
                b_hh[512:].reshape(1, LATENT).astype(np.float16))
        if use_bout:
            b_out_pad = np.zeros((1, 32), np.float32)
            b_out_pad[0, :NTOK] = b_out
            m["bout"] = np.ascontiguousarray(b_out_pad.astype(np.float16))
        in_maps.append(m)
    return in_maps


def unpack_out(o, b_core=B_CORE, n_b=N_B):
    """outT (n_packs, 128, n_b) -> logits (b_core, L, NTOK)."""
    o = np.asarray(o)
    n_chunks = b_core // n_b
    P = min(P_MAX, n_chunks)
    out = np.empty((b_core, L, NTOK), np.float32)
    i = 0
    for base in range(0, n_chunks, P):
        for t in range(L):
            for par in range(P):
                c = base + par
                out[c * n_b:(c + 1) * n_b, t, :] = \
                    o[i // 4, 32 * (i % 4):32 * (i % 4) + NTOK, :].T
                i += 1
    return out


def run(inputs, trace=False, b_core=B_CORE, mm=None):
    in_maps = make_in_maps(b_core=b_core, **inputs)
    use_bhh_n = "bhhn" in in_maps[0]
    use_bout = "bout" in in_maps[0]
    nc = _get_program(b_core, use_bhh_n, use_bout)
    core_ids = list(range(len(in_maps)))
    res = bass_utils.run_bass_kernel_spmd(nc, in_maps, core_ids, trace=trace)
    outs = [unpack_out(res.results[i]["outT"], b_core=b_core) for i in core_ids]
    return np.concatenate(outs, axis=0), res


_PROGRAM_CACHE = {}


def _get_program(b_core, use_bhh_n, use_bout):
    key = (b_core, use_bhh_n, use_bout)
    if key not in _PROGRAM_CACHE:
        _PROGRAM_CACHE[key] = build_program(
            b_core=b_core, use_bhh_n=use_bhh_n, use_bout=use_bout)
    return _PROGRAM_CACHE[key]


def kernel(**inputs) -> np.ndarray:
    out, _ = run(inputs, trace=False)
    return out
